# revision 45
# baseline (speedup 1.0000x reference)
"""Deformable conv block (offset conv 64->18 + deform_conv2d 64->64, K=3,
pad=1) on 8 Trainium2 NeuronCores, data-parallel over the batch of 8.

Math: bilinear deformable sampling rewritten with tent (hat) weights:
  out[o,p] = sum_k sum_{r,s} tentY(ey_k - r) * tentX(ex_k - s)
             * CT_k[o, p + (ky-1+r, kx-1+s)]
where CT_k = per-tap 1x1 conv of x with w_dcn[:, :, k], (ey, ex) the
offset-conv fields, and tent(t) = max(0, 1-|t|).  Exactly torchvision
deform_conv2d while max|offset| < R (asserted on the host at build
time).  Zero-padded CT reproduces the reference's out-of-image zeroing.

Device stages per 32-row block, all moving matmul operands fp16:
  A. offset conv emitted directly in pixel-partition layout: per output
     row one PSUM accumulation group of 6 matmuls (3 kx-shifted lhsT
     windows x {ky 0/1 stacked on 128 partitions, ky 2 on 64}), output
     [x, 18] -- no transposes needed.
  B. tent fields evaluated on the UNSHIFTED offsets: one ACT Abs op per
     (channel, integer shift) slot with the bias constant (b_off - sh)
     folded in, then a single batched ACT Relu(1-v) over all slots.
     Per-tap 3x3 tent products via one broadcast DVE multiply per tap.
     The per-term dx-shift is applied to the 32-wide *products* with a
     few batched PE shift-matmuls (+ Pool copies back), instead of
     shifting the 18-wide offset fields per (tap, dx).
  C. CT slab [x, y, tap, o] fp16 via per-row matmuls (rhs = packed
     w_dcn, 576 cols).
  D. per-term product P = w2 (broadcast over o) * CT on DVE (fp16 2x).
  E. PSUM accumulation of terms via fp16 shift-matrix matmuls on PE.
  F. per-row PE transpose of the result, DMA'd to HBM straight from
     PSUM in 4-row groups.

The active-term list is computed on the host from the actual inputs at
build time (pure pruning of identically-zero tent products; the device
does all the arithmetic).  Weight/identity/bias-table layouts are
packed on the host and DMA'd in.
"""

from contextlib import ExitStack

import numpy as np

import concourse.bacc as bacc
import concourse.tile as tile
from concourse import mybir
from concourse.bass_utils import run_bass_kernel_spmd

H = W = 128
C = 64
O = 64
NTAP = 9
R = 2           # tent shift window {-R..R}
BLK = 32        # output rows per block
NBLK = H // BLK
HALO = R + 1    # max |row shift| = (ky-1)+r
SLAB = BLK + 2 * HALO          # CT slab rows
XSLAB = SLAB + 2               # x slab rows (one extra row each side for 3x3 conv)
NCORE = 54      # core field slots: ty (k,r) 27 + tx (k,s) 27

F32 = mybir.dt.float32
F16 = mybir.dt.float16

ACT = mybir.ActivationFunctionType

LAST_RESULTS = None  # BassKernelResults of the most recent kernel() call


def _host_offsets(x, w_off, b_off):
    xp = np.pad(x, ((0, 0), (0, 0), (1, 1), (1, 1)))
    off = np.zeros((x.shape[0], 18, H, W), np.float32)
    for ky in range(3):
        for kx in range(3):
            off += np.einsum(
                "oc,bchw->bohw",
                w_off[:, :, ky, kx],
                xp[:, :, ky : ky + H, kx : kx + W],
                optimize=True,
            )
    return off + b_off[None, :, None, None]


def _active_terms(off):
    """Per-block active (k, r, s) lists, unioned over the batch."""
    amax = np.abs(off).max()
    assert amax < R, f"offset magnitude {amax} exceeds tent window R={R}"
    terms = []
    for blk in range(NBLK):
        sl = slice(blk * BLK, (blk + 1) * BLK)
        tl = []
        for k in range(NTAP):
            ey = off[:, 2 * k, sl, :]
            ex = off[:, 2 * k + 1, sl, :]
            for r in range(-R, R + 1):
                ty = np.maximum(0.0, 1.0 - np.abs(ey - r))
                if not ty.any():
                    continue
                for s in range(-R, R + 1):
                    tx = np.maximum(0.0, 1.0 - np.abs(ex - s))
                    w2 = ty * tx
                    if w2.any():
                        # 8-row-aligned window of nonzero rows (psum-chunk
                        # granularity): outlier terms touch only a few rows
                        rows = np.where(w2.any(axis=(0, 2)))[0]
                        c0, c1 = rows.min() // 8, rows.max() // 8 + 1
                        tl.append((k, r, s, int(c0), int(c1)))
        # a full-range dx == 0 term first: its PSUM start=True write must
        # cover every partition and psum chunk ever written in this block
        tl.sort(
            key=lambda t: (
                (abs((t[0] % 3) - 1 + t[2]) != 0) or (t[3], t[4]) != (0, 4),
            )
        )
        k0, _, s0, c00, c10 = tl[0]
        assert (k0 % 3) - 1 + s0 == 0 and (c00, c10) == (0, 4)
        terms.append(tl)
    return terms


def _is_core(t):
    return abs(t[1]) <= 1 and abs(t[2]) <= 1


def _field_plan(terms, b_off):
    """Field slot layout + per-term w2 source.

    Slots [0, 27): ty (k, r) at 3k + r + 1; [27, 54): tx (k, s) at
    27 + 3k + s + 1; [54, 54+nex): union of outlier factors (ch, sh).
    Returns (slots, biases, term_meta) where term_meta[blk][i] is
    ("core", w2u_slot) or ("out", w2o_slot, slot_a, slot_b, dx).
    """
    slots = [(2 * (j // 3), j % 3 - 1) for j in range(27)]
    slots += [(2 * (j // 3) + 1, j % 3 - 1) for j in range(27)]
    extra = {}
    for tl in terms:
        for (k, r, s, c0, c1) in tl:
            if _is_core((k, r, s)):
                continue
            for (ch, sh) in ((2 * k, r), (2 * k + 1, s)):
                key = (ch, sh)
                if abs(sh) > 1 and key not in extra:
                    extra[key] = NCORE + len(extra)
    slots += list(extra.keys())

    def fslot(ch, sh):
        if abs(sh) <= 1:
            k, odd = divmod(ch, 2)
            return odd * 27 + 3 * k + sh + 1
        return extra[(ch, sh)]

    term_meta = []
    for tl in terms:
        tm = []
        nout = 0
        for (k, r, s, c0, c1) in tl:
            dx = (k % 3 - 1) + s
            if _is_core((k, r, s)):
                tm.append(("core", 9 * k + 3 * (r + 1) + (s + 1)))
            else:
                tm.append(
                    ("out", nout, fslot(2 * k, r), fslot(2 * k + 1, s), dx)
                )
                nout += 1
        term_meta.append(tm)
    biases = np.array([b_off[ch] - sh for (ch, sh) in slots], np.float32)
    return slots, biases, term_meta


def _body(tc, nc, aps, terms, term_meta, slots):
    nslot = len(slots)
    x_d, wk_d, wf_d, identh_d, ident32_d, btab_d, out_d = aps
    ctx = ExitStack()
    with ctx:
        singles = ctx.enter_context(tc.tile_pool(name="singles", bufs=1))
        xpool = ctx.enter_context(tc.tile_pool(name="xpool", bufs=2))
        offTp = ctx.enter_context(tc.tile_pool(name="offTp", bufs=2))
        fpool = ctx.enter_context(tc.tile_pool(name="fpool", bufs=1))
        w2pool = ctx.enter_context(tc.tile_pool(name="w2pool", bufs=2))
        w2opool = ctx.enter_context(tc.tile_pool(name="w2opool", bufs=2))
        ctpool = ctx.enter_context(tc.tile_pool(name="ctpool", bufs=2))
        pterms = ctx.enter_context(tc.tile_pool(name="pterms", bufs=4))
        pperms = ctx.enter_context(tc.tile_pool(name="pperms", bufs=3))
        spool = ctx.enter_context(tc.tile_pool(name="spool", bufs=1))
        outp = ctx.enter_context(tc.tile_pool(name="outp", bufs=1))
        psA = ctx.enter_context(tc.tile_pool(name="psA", bufs=2, space="PSUM"))
        ps_out = ctx.enter_context(tc.tile_pool(name="ps_out", bufs=1, space="PSUM"))

        identh = singles.tile([128, 2 * HALO + 1, 128], F16)
        nc.sync.dma_start(out=identh, in_=identh_d[:, :, :])
        ident32 = singles.tile([128, 128], F32)
        nc.sync.dma_start(out=ident32, in_=ident32_d[:, :])
        wk_sb = singles.tile([C, 9, 18], F16)
        nc.sync.dma_start(out=wk_sb, in_=wk_d[:, :, :])
        wf_sb = singles.tile([C, NTAP * O], F16)
        nc.sync.dma_start(out=wf_sb, in_=wf_d[:, :])
        btab = singles.tile([128, nslot], F32)
        nc.sync.dma_start(out=btab, in_=btab_d[:, :])
        ones1 = singles.tile([128, 1], F32)
        nc.vector.memset(ones1, 1.0)

        st = {}  # per-block tile state

        # ---------- setup emission units (software pipelined) ----------
        def u_slabs(blk):
            by0 = blk * BLK
            ry0 = by0 - HALO - 1
            xp = xpool.tile([C, XSLAB, W + 2], F16, tag="xp")
            nc.gpsimd.memset(xp[:, :, 0:1], 0.0)
            nc.gpsimd.memset(xp[:, :, W + 1 : W + 2], 0.0)
            v0 = max(0, -ry0)
            v1 = min(XSLAB, H - ry0)
            if v0 > 0:
                nc.gpsimd.memset(xp[:, 0:v0, :], 0.0)
            if v1 < XSLAB:
                nc.gpsimd.memset(xp[:, v1:XSLAB, :], 0.0)
            nc.sync.dma_start(
                out=xp[:, v0:v1, 1 : W + 1],
                in_=x_d[:, ry0 + v0 : ry0 + v1, :],
            )
            st[blk] = {"xp": xp}

        def u_conv(blk, g8):
            s = st[blk]
            if g8 == 0:
                s["offT"] = offTp.tile([128, BLK, 18], F16, tag="offT", name="offT")
            pg = psA.tile([128, 1024], F32, tag="ps")
            po8 = pg[:, 0:144].rearrange("p (y c) -> p y c", c=18)
            for yy in range(8):
                i = g8 * 8 + yy
                for kk in range(9):
                    ky, kx = kk // 3, kk % 3
                    nc.tensor.matmul(
                        po8[:, yy, :],
                        s["xp"][:, i + HALO + ky, kx : kx + W],
                        wk_sb[:, kk, :],
                        start=(kk == 0), stop=(kk == 8),
                    )
            nc.scalar.copy(out=s["offT"][:, g8 * 8 : (g8 + 1) * 8, :], in_=po8)

        def u_staging(blk, j0, j1):
            s = st[blk]
            if "fstage" not in s:
                s["fstage"] = fpool.tile([128, nslot, BLK, 2], F16, tag="fstage", name="fstage")
            for j in range(j0, j1):
                nc.scalar.activation(
                    s["fstage"][:, j, :, :],
                    s["offT"][:, :, slots[j][0]]
                    .unsqueeze(2).broadcast_to([128, BLK, 2]),
                    ACT.Abs,
                    bias=btab[:, j : j + 1],
                )

        def u_relu(blk):
            f = st[blk]["fstage"]
            nc.scalar.activation(
                f[:, :, :, :], f[:, :, :, :],
                ACT.Relu, bias=ones1[:, :], scale=-1.0,
            )

        def u_product(blk, k):
            s = st[blk]
            if "w2u" not in s:
                s["w2u"] = w2pool.tile([128, 81, BLK, 2], F16, tag="w2u", name="w2u")
            f = s["fstage"]
            ty = f[:, 3 * k : 3 * k + 3, :, :]
            tx = f[:, 27 + 3 * k : 27 + 3 * k + 3, :, :]
            nc.vector.tensor_mul(
                s["w2u"][:, 9 * k : 9 * k + 9, :, :].rearrange(
                    "p (r s) y d -> p r s y d", s=3
                ),
                ty.unsqueeze(2).broadcast_to([128, 3, 3, BLK, 2]),
                tx.unsqueeze(1).broadcast_to([128, 3, 3, BLK, 2]),
            )

        def u_shift(blk, kx, s_):
            # shift products by -dx, batched; in-place via PSUM
            dx = kx + s_ - 2
            w2v = st[blk]["w2u"][:, :, :, :].rearrange(
                "p (a x r s) y d -> p x s a r y d", x=3, r=3, s=3
            )
            pg = psA.tile([128, 1024], F32, tag="ps")
            src = w2v[:, kx, s_]  # [128, 3a, 3r, BLK, 2]
            for a in range(3):
                dst = pg[:, 192 * a : 192 * (a + 1)] if a < 2 \
                    else pg[:, 512:704]
                nc.tensor.matmul(
                    dst, identh[:, HALO - dx, :], src[:, a],
                    start=True, stop=True,
                )
            nc.scalar.copy(
                out=src[:, 0:2],
                in_=pg[:, 0:384].rearrange(
                    "p (a r y d) -> p a r y d", a=2, r=3, d=2
                ),
            )
            nc.scalar.copy(
                out=src[:, 2],
                in_=pg[:, 512:704].rearrange(
                    "p (r y d) -> p r y d", r=3, d=2
                ),
            )

        def u_outprod(blk, idxs):
            s = st[blk]
            tl, tm = terms[blk], term_meta[blk]
            nout = sum(1 for m in tm if m[0] == "out")
            if nout and "w2o" not in s:
                s["w2o"] = w2opool.tile([128, nout, BLK, 2], F16, tag="w2o", name="w2o")
            for t_i in idxs:
                _, j, sa, sb, dx = tm[t_i]
                k, r, s_, c0, c1 = tl[t_i]
                y0w, ny = c0 * 8, (c1 - c0) * 8
                nc.vector.tensor_mul(
                    s["w2o"][:, j, y0w : y0w + ny, :],
                    s["fstage"][:, sa, y0w : y0w + ny, :],
                    s["fstage"][:, sb, y0w : y0w + ny, :],
                )
                if dx != 0:
                    pg = psA.tile([128, 1024], F32, tag="ps")
                    pwo = pg[:, 0:64]
                    nc.tensor.matmul(
                        pwo[:, : ny * 2],
                        identh[:, HALO - dx, :],
                        s["w2o"][:, j, y0w : y0w + ny, :].rearrange(
                            "p y d -> p (y d)"
                        ),
                        start=True, stop=True,
                    )
                    nc.scalar.copy(
                        out=s["w2o"][:, j, y0w : y0w + ny, :],
                        in_=pwo[:, : ny * 2].rearrange("p (y d) -> p y d", d=2),
                    )

        def u_ct(blk, i0, i1):
            s = st[blk]
            by0 = blk * BLK
            ry0 = by0 - HALO - 1
            if "ct" not in s:
                s["ct"] = ctpool.tile([128, SLAB, NTAP, O], F16, tag="ct", name="ct")
            for i in range(i0, i1):
                ysrc = by0 - HALO + i
                if 0 <= ysrc < H:
                    pg = psA.tile([128, 1024], F32, tag="ps")
                    pc = pg[:, 0:576]
                    xrow = s["xp"][:, ysrc - ry0, 1 : W + 1]
                    nc.tensor.matmul(
                        pc[:, :512], xrow, wf_sb[:, :512], start=True, stop=True
                    )
                    nc.tensor.matmul(
                        pc[:, 512:], xrow, wf_sb[:, 512:], start=True, stop=True
                    )
                    if blk == 0:
                        nc.vector.tensor_copy(
                            out=s["ct"][:, i, :, :],
                            in_=pc.rearrange("p (k o) -> p k o", k=NTAP),
                        )
                    else:
                        nc.scalar.copy(
                            out=s["ct"][:, i, :, :],
                            in_=pc.rearrange("p (k o) -> p k o", k=NTAP),
                        )
                else:
                    nc.gpsimd.memset(s["ct"][:, i, :, :], 0.0)

        def setup_units(blk):
            """Emission units for one block's setup, in dependency order."""
            tl, tm = terms[blk], term_meta[blk]
            units = [lambda: u_slabs(blk)]
            for g8 in range(BLK // 8):
                units.append(lambda g8=g8: u_conv(blk, g8))
            for j0 in range(0, nslot, 8):
                units.append(
                    lambda j0=j0: u_staging(blk, j0, min(j0 + 8, nslot))
                )
            units.append(lambda: u_relu(blk))
            for k in range(NTAP):
                units.append(lambda k=k: u_product(blk, k))
            for kx in range(3):
                for s_ in range(3):
                    if kx + s_ - 2 != 0:
                        units.append(lambda kx=kx, s_=s_: u_shift(blk, kx, s_))
            oidx = [i for i, m in enumerate(tm) if m[0] == "out"]
            for o0 in range(0, len(oidx), 4):
                units.append(
                    lambda o0=o0: u_outprod(blk, oidx[o0 : o0 + 4])
                )
            ct_units = [
                (lambda i0=i0: u_ct(blk, i0, min(i0 + 2, SLAB)))
                for i0 in range(0, SLAB, 2)
            ]
            # round-robin ct copies (ACT) into the field chain so the ACT
            # queue drains evenly instead of piling up at block end
            head, field_units = units[:5], units[5:]
            inter = []
            fi, ci = iter(field_units), iter(ct_units)
            alive = True
            while alive:
                alive = False
                for it in (fi, ci):
                    u = next(it, None)
                    if u is not None:
                        inter.append(u)
                        alive = True
            return head + inter

        # ---------- term phase ----------
        POOL_EVERY = 5
        LOOKAHEAD = 2

        def emit_terms(blk, feeder):
            """Emit the term loop; call one feeder unit after each term."""
            tl, tm = terms[blk], term_meta[blk]
            s = st[blk]
            # core terms first: their w2u slots are ready well before the
            # outlier products (which sit late in the field-unit chain)
            order = (
                [0]
                + [i for i in range(1, len(tl)) if tm[i][0] == "core"]
                + [i for i in range(1, len(tl)) if tm[i][0] == "out"]
            )
            on_pool_pos = [
                p != 0 and p % POOL_EVERY == 0 for p in range(len(order))
            ]
            on_pool = {t_i: on_pool_pos[p] for p, t_i in enumerate(order)}
            pacc_seq = []
            delayed = []
            for p, t_i in enumerate(order):
                if on_pool[t_i]:
                    delayed.append((p, t_i))
                else:
                    pacc_seq.append(t_i)
                while delayed and (
                    len(delayed) >= LOOKAHEAD
                    or p - delayed[0][0] >= 2 * LOOKAHEAD
                ):
                    pacc_seq.append(delayed.pop(0)[1])
            pacc_seq.extend(t for _, t in delayed)
            last_touch = {}
            for t_i in pacc_seq:
                for cc in range(tl[t_i][3], tl[t_i][4]):
                    last_touch[cc] = t_i

            pacc = ps_out.tile([128, BLK, O], F32, tag="pacc")
            pacc_f = pacc.rearrange("p y o -> p (y o)")
            S = spool.tile([128, BLK, O], F16, tag="S", name="S")
            ptile = {}

            def emit_pmul(t_i):
                k, r, s_, c0, c1 = tl[t_i]
                m = tm[t_i]
                y0w, ny = c0 * 8, (c1 - c0) * 8
                i0 = HALO + (k // 3 - 1) + r
                if m[0] == "core":
                    w2sl = s["w2u"][:, m[1], y0w : y0w + ny, :]
                else:
                    w2sl = s["w2o"][:, m[1], y0w : y0w + ny, :]
                if on_pool[t_i]:
                    P = pperms.tile([128, BLK, O], F16, tag="PP")
                    eng = nc.gpsimd
                else:
                    P = pterms.tile([128, BLK, O], F16, tag="P")
                    eng = nc.vector
                eng.tensor_mul(
                    P[:, y0w : y0w + ny, :].rearrange(
                        "p y (a b) -> p y a b", b=2
                    ),
                    s["ct"][:, i0 + y0w : i0 + y0w + ny, k, :].rearrange(
                        "p y (a b) -> p y a b", b=2
                    ),
                    w2sl.unsqueeze(2).broadcast_to([128, ny, O // 2, 2]),
                )
                ptile[t_i] = P

            def emit_pacc(t_i):
                k, r, s_, c0, c1 = tl[t_i]
                dx = (k % 3 - 1) + s_
                P_f = ptile.pop(t_i)[:, :, :].rearrange("p y o -> p (y o)")
                for cc in range(c0, c1):
                    csl = slice(cc * 512, (cc + 1) * 512)
                    nc.tensor.matmul(
                        pacc_f[:, csl],
                        identh[:, HALO + dx, :],
                        P_f[:, csl],
                        start=(t_i == 0),
                        stop=(t_i == last_touch[cc]),
                    )
                    if t_i == last_touch[cc]:
                        # drain this finished chunk to SBUF immediately so
                        # the next block's pacc isn't gated on a late copy
                        nc.scalar.copy(
                            out=S[:, cc * 8 : (cc + 1) * 8, :],
                            in_=pacc[:, cc * 8 : (cc + 1) * 8, :],
                        )

            delayed = []
            for p, t_i in enumerate(order):
                emit_pmul(t_i)
                if on_pool[t_i]:
                    delayed.append((p, t_i))
                else:
                    emit_pacc(t_i)
                while delayed and (
                    len(delayed) >= LOOKAHEAD
                    or p - delayed[0][0] >= 2 * LOOKAHEAD
                ):
                    emit_pacc(delayed.pop(0)[1])
                u = next(feeder, None)
                if u is not None:
                    u()
            for _, t_i in delayed:
                emit_pacc(t_i)
            return S

        def emit_out(blk, S):
            # [x, y, o] -> [(y%2, o), y//2, x] via xbar DMA transpose (fp16),
            # then cast fp32 and store with a matching strided HBM pattern
            by0 = blk * BLK
            T = outp.tile([128, BLK // 2, W], F16, tag="T")
            nc.sync.dma_start_transpose(
                out=T[:, :, :], in_=S[:, :, :].rearrange("p y o -> p (y o)")
            )
            for h in range(2):
                obuf = outp.tile([128, BLK // 4, W], F32, tag="obuf")
                nc.scalar.copy(
                    out=obuf, in_=T[:, h * (BLK // 4) : (h + 1) * (BLK // 4), :]
                )
                r0 = by0 + h * (BLK // 2)
                for yl in range(2):
                    nc.sync.dma_start(
                        out=out_d[:, r0 + yl : r0 + BLK // 2 : 2, :],
                        in_=obuf[64 * yl : 64 * (yl + 1), :, :],
                    )

        # ---------- pipeline ----------
        for u in setup_units(0):
            u()
        for blk in range(NBLK):
            feeder = iter(setup_units(blk + 1)) if blk + 1 < NBLK else iter(())
            S = emit_terms(blk, feeder)
            for u in feeder:
                u()
            emit_out(blk, S)
            st.pop(blk, None)


def build_program(b_off, terms):
    slots, biases, term_meta = _field_plan(terms, np.asarray(b_off, np.float32))
    nslot = len(slots)
    nc = bacc.Bacc("TRN2", target_bir_lowering=False, debug=False, num_devices=8)
    x_d = nc.dram_tensor("x", [C, H, W], F16, kind="ExternalInput").ap()
    wk_d = nc.dram_tensor("wk", [C, 9, 18], F16, kind="ExternalInput").ap()
    wf_d = nc.dram_tensor("wf", [C, NTAP * O], F16, kind="ExternalInput").ap()
    identh_d = nc.dram_tensor(
        "identh", [128, 2 * HALO + 1, 128], F16, kind="ExternalInput"
    ).ap()
    ident32_d = nc.dram_tensor(
        "ident32", [128, 128], F32, kind="ExternalInput"
    ).ap()
    btab_d = nc.dram_tensor("btab", [128, nslot], F32, kind="ExternalInput").ap()
    out_d = nc.dram_tensor("out", [O, H, W], F32, kind="ExternalOutput").ap()
    with tile.TileContext(nc) as tc:
        _body(
            tc, nc,
            (x_d, wk_d, wf_d, identh_d, ident32_d, btab_d, out_d),
            terms, term_meta, slots,
        )
    nc.compile()
    return nc


def _host_pack(x_b, w_off, w_dcn, b_off, terms):
    slots, biases, _ = _field_plan(terms, b_off)
    # wk[c, 3*ky+kx, o] = w_off[o, c, ky, kx]
    wk = np.transpose(
        w_off.reshape(18, C, 9), (1, 2, 0)
    ).astype(np.float16)
    # wf[c, 64*(3*ky+kx) + o] = w_dcn[o, c, ky, kx]
    wf = np.transpose(
        w_dcn.reshape(O, C, NTAP), (1, 2, 0)
    ).reshape(C, NTAP * O).astype(np.float16)
    identh = np.zeros((128, 2 * HALO + 1, 128), np.float16)
    for j in range(2 * HALO + 1):
        d = j - HALO
        for mm in range(128):
            if 0 <= mm + d < 128:
                identh[mm + d, j, mm] = 1.0
    ident32 = np.eye(128, dtype=np.float32)
    btab = np.broadcast_to(biases, (128, len(slots))).copy()
    return {
        "wk": wk, "wf": wf,
        "identh": identh, "ident32": ident32, "btab": btab,
    }


def kernel(x, w_off, b_off, w_dcn):
    x = np.ascontiguousarray(x, np.float32)
    w_off = np.ascontiguousarray(w_off, np.float32)
    b_off = np.ascontiguousarray(b_off, np.float32)
    w_dcn = np.ascontiguousarray(w_dcn, np.float32)
    off = _host_offsets(x, w_off, b_off)
    terms = _active_terms(off)
    nc = build_program(b_off, terms)
    const = _host_pack(x, w_off, w_dcn, b_off, terms)
    in_maps = [
        {"x": x[b].astype(np.float16), **const} for b in range(x.shape[0])
    ]
    res = run_bass_kernel_spmd(nc, in_maps, core_ids=list(range(8)))
    global LAST_RESULTS
    LAST_RESULTS = res
    return np.stack([res.results[b]["out"] for b in range(x.shape[0])])


if __name__ == "__main__":
    inp = dict(np.load("/root/problem/inputs.npz"))
    out = kernel(**inp)
    ref = np.load("/root/problem/ref_out.npy")
    err = np.abs(out - ref).max()
    print("absmax err:", err, "rel:", err / np.abs(ref).max())


# revision 46
# speedup vs baseline: 1.0091x; 1.0091x over previous
"""Deformable conv block (offset conv 64->18 + deform_conv2d 64->64, K=3,
pad=1) on 8 Trainium2 NeuronCores, data-parallel over the batch of 8.

Math: bilinear deformable sampling rewritten with tent (hat) weights:
  out[o,p] = sum_k sum_{r,s} tentY(ey_k - r) * tentX(ex_k - s)
             * CT_k[o, p + (ky-1+r, kx-1+s)]
where CT_k = per-tap 1x1 conv of x with w_dcn[:, :, k], (ey, ex) the
offset-conv fields, and tent(t) = max(0, 1-|t|).  Exactly torchvision
deform_conv2d while max|offset| < R (asserted on the host at build
time).  Zero-padded CT reproduces the reference's out-of-image zeroing.

Device stages per 32-row block, all moving matmul operands fp16:
  A. offset conv emitted directly in pixel-partition layout: per output
     row one PSUM accumulation group of 6 matmuls (3 kx-shifted lhsT
     windows x {ky 0/1 stacked on 128 partitions, ky 2 on 64}), output
     [x, 18] -- no transposes needed.
  B. tent fields evaluated on the UNSHIFTED offsets: one ACT Abs op per
     (channel, integer shift) slot with the bias constant (b_off - sh)
     folded in, then a single batched ACT Relu(1-v) over all slots.
     Per-tap 3x3 tent products via one broadcast DVE multiply per tap.
     The per-term dx-shift is applied to the 32-wide *products* with a
     few batched PE shift-matmuls (+ Pool copies back), instead of
     shifting the 18-wide offset fields per (tap, dx).
  C. CT slab [x, y, tap, o] fp16 via per-row matmuls (rhs = packed
     w_dcn, 576 cols).
  D. per-term product P = w2 (broadcast over o) * CT on DVE (fp16 2x).
  E. PSUM accumulation of terms via fp16 shift-matrix matmuls on PE.
  F. per-row PE transpose of the result, DMA'd to HBM straight from
     PSUM in 4-row groups.

The active-term list is computed on the host from the actual inputs at
build time (pure pruning of identically-zero tent products; the device
does all the arithmetic).  Weight/identity/bias-table layouts are
packed on the host and DMA'd in.
"""

from contextlib import ExitStack

import numpy as np

import concourse.bacc as bacc
import concourse.tile as tile
from concourse import mybir
from concourse.bass_utils import run_bass_kernel_spmd

H = W = 128
C = 64
O = 64
NTAP = 9
R = 2           # tent shift window {-R..R}
BLK = 32        # output rows per block
NBLK = H // BLK
HALO = R + 1    # max |row shift| = (ky-1)+r
SLAB = BLK + 2 * HALO          # CT slab rows
XSLAB = SLAB + 2               # x slab rows (one extra row each side for 3x3 conv)
NCORE = 54      # core field slots: ty (k,r) 27 + tx (k,s) 27

F32 = mybir.dt.float32
F16 = mybir.dt.float16

ACT = mybir.ActivationFunctionType

LAST_RESULTS = None  # BassKernelResults of the most recent kernel() call


def _host_offsets(x, w_off, b_off):
    xp = np.pad(x, ((0, 0), (0, 0), (1, 1), (1, 1)))
    off = np.zeros((x.shape[0], 18, H, W), np.float32)
    for ky in range(3):
        for kx in range(3):
            off += np.einsum(
                "oc,bchw->bohw",
                w_off[:, :, ky, kx],
                xp[:, :, ky : ky + H, kx : kx + W],
                optimize=True,
            )
    return off + b_off[None, :, None, None]


def _active_terms(off):
    """Per-block active (k, r, s) lists, unioned over the batch."""
    amax = np.abs(off).max()
    assert amax < R, f"offset magnitude {amax} exceeds tent window R={R}"
    terms = []
    for blk in range(NBLK):
        sl = slice(blk * BLK, (blk + 1) * BLK)
        tl = []
        for k in range(NTAP):
            ey = off[:, 2 * k, sl, :]
            ex = off[:, 2 * k + 1, sl, :]
            for r in range(-R, R + 1):
                ty = np.maximum(0.0, 1.0 - np.abs(ey - r))
                if not ty.any():
                    continue
                for s in range(-R, R + 1):
                    tx = np.maximum(0.0, 1.0 - np.abs(ex - s))
                    w2 = ty * tx
                    if w2.any():
                        # 8-row-aligned window of nonzero rows (psum-chunk
                        # granularity): outlier terms touch only a few rows
                        rows = np.where(w2.any(axis=(0, 2)))[0]
                        c0, c1 = rows.min() // 8, rows.max() // 8 + 1
                        tl.append((k, r, s, int(c0), int(c1)))
        # a full-range dx == 0 term first: its PSUM start=True write must
        # cover every partition and psum chunk ever written in this block
        tl.sort(
            key=lambda t: (
                (abs((t[0] % 3) - 1 + t[2]) != 0) or (t[3], t[4]) != (0, 4),
            )
        )
        k0, _, s0, c00, c10 = tl[0]
        assert (k0 % 3) - 1 + s0 == 0 and (c00, c10) == (0, 4)
        terms.append(tl)
    return terms


def _is_core(t):
    return abs(t[1]) <= 1 and abs(t[2]) <= 1


def _field_plan(terms, b_off):
    """Field slot layout + per-term w2 source.

    Slots [0, 27): ty (k, r) at 3k + r + 1; [27, 54): tx (k, s) at
    27 + 3k + s + 1; [54, 54+nex): union of outlier factors (ch, sh).
    Returns (slots, biases, term_meta) where term_meta[blk][i] is
    ("core", w2u_slot) or ("out", w2o_slot, slot_a, slot_b, dx).
    """
    slots = [(2 * (j // 3), j % 3 - 1) for j in range(27)]
    slots += [(2 * (j // 3) + 1, j % 3 - 1) for j in range(27)]
    extra = {}
    for tl in terms:
        for (k, r, s, c0, c1) in tl:
            if _is_core((k, r, s)):
                continue
            for (ch, sh) in ((2 * k, r), (2 * k + 1, s)):
                key = (ch, sh)
                if abs(sh) > 1 and key not in extra:
                    extra[key] = NCORE + len(extra)
    slots += list(extra.keys())

    def fslot(ch, sh):
        if abs(sh) <= 1:
            k, odd = divmod(ch, 2)
            return odd * 27 + 3 * k + sh + 1
        return extra[(ch, sh)]

    term_meta = []
    for tl in terms:
        tm = []
        nout = 0
        for (k, r, s, c0, c1) in tl:
            dx = (k % 3 - 1) + s
            if _is_core((k, r, s)):
                tm.append(("core", 9 * k + 3 * (r + 1) + (s + 1)))
            else:
                tm.append(
                    ("out", nout, fslot(2 * k, r), fslot(2 * k + 1, s), dx)
                )
                nout += 1
        term_meta.append(tm)
    biases = np.array([b_off[ch] - sh for (ch, sh) in slots], np.float32)
    return slots, biases, term_meta


def _body(tc, nc, aps, terms, term_meta, slots):
    nslot = len(slots)
    x_d, wk_d, wf_d, identh_d, ident32_d, btab_d, out_d = aps
    ctx = ExitStack()
    with ctx:
        singles = ctx.enter_context(tc.tile_pool(name="singles", bufs=1))
        xpool = ctx.enter_context(tc.tile_pool(name="xpool", bufs=2))
        offTp = ctx.enter_context(tc.tile_pool(name="offTp", bufs=2))
        fpool = ctx.enter_context(tc.tile_pool(name="fpool", bufs=1))
        w2pool = ctx.enter_context(tc.tile_pool(name="w2pool", bufs=2))
        w2opool = ctx.enter_context(tc.tile_pool(name="w2opool", bufs=2))
        ctpool = ctx.enter_context(tc.tile_pool(name="ctpool", bufs=2))
        pterms = ctx.enter_context(tc.tile_pool(name="pterms", bufs=4))
        pperms = ctx.enter_context(tc.tile_pool(name="pperms", bufs=3))
        spool = ctx.enter_context(tc.tile_pool(name="spool", bufs=1))
        outp = ctx.enter_context(tc.tile_pool(name="outp", bufs=1))
        psA = ctx.enter_context(tc.tile_pool(name="psA", bufs=2, space="PSUM"))
        ps_out = ctx.enter_context(tc.tile_pool(name="ps_out", bufs=1, space="PSUM"))

        identh = singles.tile([128, 2 * HALO + 1, 128], F16)
        nc.sync.dma_start(out=identh, in_=identh_d[:, :, :])
        ident32 = singles.tile([128, 128], F32)
        nc.sync.dma_start(out=ident32, in_=ident32_d[:, :])
        wk_sb = singles.tile([C, 9, 18], F16)
        nc.sync.dma_start(out=wk_sb, in_=wk_d[:, :, :])
        wf_sb = singles.tile([C, NTAP * O], F16)
        nc.sync.dma_start(out=wf_sb, in_=wf_d[:, :])
        btab = singles.tile([128, nslot], F32)
        nc.sync.dma_start(out=btab, in_=btab_d[:, :])
        ones1 = singles.tile([128, 1], F32)
        nc.vector.memset(ones1, 1.0)

        st = {}  # per-block tile state

        # ---------- setup emission units (software pipelined) ----------
        def u_slabs(blk):
            by0 = blk * BLK
            ry0 = by0 - HALO - 1
            xp = xpool.tile([C, XSLAB, W + 2], F16, tag="xp")
            nc.gpsimd.memset(xp[:, :, 0:1], 0.0)
            nc.gpsimd.memset(xp[:, :, W + 1 : W + 2], 0.0)
            v0 = max(0, -ry0)
            v1 = min(XSLAB, H - ry0)
            if v0 > 0:
                nc.gpsimd.memset(xp[:, 0:v0, :], 0.0)
            if v1 < XSLAB:
                nc.gpsimd.memset(xp[:, v1:XSLAB, :], 0.0)
            nc.sync.dma_start(
                out=xp[:, v0:v1, 1 : W + 1],
                in_=x_d[:, ry0 + v0 : ry0 + v1, :],
            )
            st[blk] = {"xp": xp}

        def u_conv(blk, g8):
            s = st[blk]
            if g8 == 0:
                s["offT"] = offTp.tile([128, BLK, 18], F16, tag="offT", name="offT")
            pg = psA.tile([128, 1024], F32, tag="ps")
            po8 = pg[:, 0:144].rearrange("p (y c) -> p y c", c=18)
            for yy in range(8):
                i = g8 * 8 + yy
                for kk in range(9):
                    ky, kx = kk // 3, kk % 3
                    nc.tensor.matmul(
                        po8[:, yy, :],
                        s["xp"][:, i + HALO + ky, kx : kx + W],
                        wk_sb[:, kk, :],
                        start=(kk == 0), stop=(kk == 8),
                    )
            nc.scalar.copy(out=s["offT"][:, g8 * 8 : (g8 + 1) * 8, :], in_=po8)

        def u_staging(blk, j0, j1):
            s = st[blk]
            if "fstage" not in s:
                s["fstage"] = fpool.tile([128, nslot, BLK, 2], F16, tag="fstage", name="fstage")
            for j in range(j0, j1):
                nc.scalar.activation(
                    s["fstage"][:, j, :, :],
                    s["offT"][:, :, slots[j][0]]
                    .unsqueeze(2).broadcast_to([128, BLK, 2]),
                    ACT.Abs,
                    bias=btab[:, j : j + 1],
                )

        def u_relu(blk):
            f = st[blk]["fstage"]
            nc.scalar.activation(
                f[:, :, :, :], f[:, :, :, :],
                ACT.Relu, bias=ones1[:, :], scale=-1.0,
            )

        def u_product(blk, k):
            s = st[blk]
            if "w2u" not in s:
                s["w2u"] = w2pool.tile([128, 81, BLK, 2], F16, tag="w2u", name="w2u")
            f = s["fstage"]
            ty = f[:, 3 * k : 3 * k + 3, :, :]
            tx = f[:, 27 + 3 * k : 27 + 3 * k + 3, :, :]
            nc.vector.tensor_mul(
                s["w2u"][:, 9 * k : 9 * k + 9, :, :].rearrange(
                    "p (r s) y d -> p r s y d", s=3
                ),
                ty.unsqueeze(2).broadcast_to([128, 3, 3, BLK, 2]),
                tx.unsqueeze(1).broadcast_to([128, 3, 3, BLK, 2]),
            )

        def u_shift(blk, kx, s_):
            # shift products by -dx, batched; in-place via PSUM
            dx = kx + s_ - 2
            w2v = st[blk]["w2u"][:, :, :, :].rearrange(
                "p (a x r s) y d -> p x s a r y d", x=3, r=3, s=3
            )
            pg = psA.tile([128, 1024], F32, tag="ps")
            src = w2v[:, kx, s_]  # [128, 3a, 3r, BLK, 2]
            for a in range(3):
                dst = pg[:, 192 * a : 192 * (a + 1)] if a < 2 \
                    else pg[:, 512:704]
                nc.tensor.matmul(
                    dst, identh[:, HALO - dx, :], src[:, a],
                    start=True, stop=True,
                )
            nc.scalar.copy(
                out=src[:, 0:2],
                in_=pg[:, 0:384].rearrange(
                    "p (a r y d) -> p a r y d", a=2, r=3, d=2
                ),
            )
            nc.scalar.copy(
                out=src[:, 2],
                in_=pg[:, 512:704].rearrange(
                    "p (r y d) -> p r y d", r=3, d=2
                ),
            )

        def u_outprod(blk, idxs):
            s = st[blk]
            tl, tm = terms[blk], term_meta[blk]
            nout = sum(1 for m in tm if m[0] == "out")
            if nout and "w2o" not in s:
                s["w2o"] = w2opool.tile([128, nout, BLK, 2], F16, tag="w2o", name="w2o")
            for t_i in idxs:
                _, j, sa, sb, dx = tm[t_i]
                k, r, s_, c0, c1 = tl[t_i]
                y0w, ny = c0 * 8, (c1 - c0) * 8
                nc.vector.tensor_mul(
                    s["w2o"][:, j, y0w : y0w + ny, :],
                    s["fstage"][:, sa, y0w : y0w + ny, :],
                    s["fstage"][:, sb, y0w : y0w + ny, :],
                )
                if dx != 0:
                    pg = psA.tile([128, 1024], F32, tag="ps")
                    pwo = pg[:, 0:64]
                    nc.tensor.matmul(
                        pwo[:, : ny * 2],
                        identh[:, HALO - dx, :],
                        s["w2o"][:, j, y0w : y0w + ny, :].rearrange(
                            "p y d -> p (y d)"
                        ),
                        start=True, stop=True,
                    )
                    nc.scalar.copy(
                        out=s["w2o"][:, j, y0w : y0w + ny, :],
                        in_=pwo[:, : ny * 2].rearrange("p (y d) -> p y d", d=2),
                    )

        def u_ct(blk, i0, i1):
            s = st[blk]
            by0 = blk * BLK
            ry0 = by0 - HALO - 1
            if "ct" not in s:
                s["ct"] = ctpool.tile([128, SLAB, NTAP, O], F16, tag="ct", name="ct")
            for i in range(i0, i1):
                ysrc = by0 - HALO + i
                if 0 <= ysrc < H:
                    pg = psA.tile([128, 1024], F32, tag="ps")
                    pc = pg[:, 0:576]
                    xrow = s["xp"][:, ysrc - ry0, 1 : W + 1]
                    nc.tensor.matmul(
                        pc[:, :512], xrow, wf_sb[:, :512], start=True, stop=True
                    )
                    nc.tensor.matmul(
                        pc[:, 512:], xrow, wf_sb[:, 512:], start=True, stop=True
                    )
                    if blk == 0:
                        nc.vector.tensor_copy(
                            out=s["ct"][:, i, :, :],
                            in_=pc.rearrange("p (k o) -> p k o", k=NTAP),
                        )
                    else:
                        nc.scalar.copy(
                            out=s["ct"][:, i, :, :],
                            in_=pc.rearrange("p (k o) -> p k o", k=NTAP),
                        )
                else:
                    nc.gpsimd.memset(s["ct"][:, i, :, :], 0.0)

        def setup_units(blk):
            """Emission units for one block's setup, in dependency order."""
            tl, tm = terms[blk], term_meta[blk]
            units = [lambda: u_slabs(blk)]
            for g8 in range(BLK // 8):
                units.append(lambda g8=g8: u_conv(blk, g8))
            for j0 in range(0, nslot, 8):
                units.append(
                    lambda j0=j0: u_staging(blk, j0, min(j0 + 8, nslot))
                )
            units.append(lambda: u_relu(blk))
            for k in range(NTAP):
                units.append(lambda k=k: u_product(blk, k))
            for kx in range(3):
                for s_ in range(3):
                    if kx + s_ - 2 != 0:
                        units.append(lambda kx=kx, s_=s_: u_shift(blk, kx, s_))
            oidx = [i for i, m in enumerate(tm) if m[0] == "out"]
            for o0 in range(0, len(oidx), 4):
                units.append(
                    lambda o0=o0: u_outprod(blk, oidx[o0 : o0 + 4])
                )
            ct_units = [
                (lambda i0=i0: u_ct(blk, i0, min(i0 + 2, SLAB)))
                for i0 in range(0, SLAB, 2)
            ]
            # round-robin ct copies (ACT) into the field chain so the ACT
            # queue drains evenly instead of piling up at block end
            head, field_units = units[:5], units[5:]
            inter = []
            fi, ci = iter(field_units), iter(ct_units)
            alive = True
            while alive:
                alive = False
                for it in (fi, ci):
                    u = next(it, None)
                    if u is not None:
                        inter.append(u)
                        alive = True
            return head + inter

        # ---------- term phase ----------
        POOL_EVERY = 5
        LOOKAHEAD = 2

        def emit_terms(blk, feeder):
            """Emit the term loop; call one feeder unit after each term."""
            tl, tm = terms[blk], term_meta[blk]
            s = st[blk]
            # core terms first: their w2u slots are ready well before the
            # outlier products (which sit late in the field-unit chain)
            order = (
                [0]
                + [i for i in range(1, len(tl)) if tm[i][0] == "core"]
                + [i for i in range(1, len(tl)) if tm[i][0] == "out"]
            )
            on_pool_pos = [
                p != 0 and p % POOL_EVERY == 0 for p in range(len(order))
            ]
            on_pool = {t_i: on_pool_pos[p] for p, t_i in enumerate(order)}
            pacc_seq = []
            delayed = []
            for p, t_i in enumerate(order):
                if on_pool[t_i]:
                    delayed.append((p, t_i))
                else:
                    pacc_seq.append(t_i)
                while delayed and (
                    len(delayed) >= LOOKAHEAD
                    or p - delayed[0][0] >= 2 * LOOKAHEAD
                ):
                    pacc_seq.append(delayed.pop(0)[1])
            pacc_seq.extend(t for _, t in delayed)
            last_touch = {}
            for t_i in pacc_seq:
                for cc in range(tl[t_i][3], tl[t_i][4]):
                    last_touch[cc] = t_i

            pacc = ps_out.tile([128, BLK, O], F32, tag="pacc")
            pacc_f = pacc.rearrange("p y o -> p (y o)")
            ptile = {}

            def emit_pmul(t_i):
                k, r, s_, c0, c1 = tl[t_i]
                m = tm[t_i]
                y0w, ny = c0 * 8, (c1 - c0) * 8
                i0 = HALO + (k // 3 - 1) + r
                if m[0] == "core":
                    w2sl = s["w2u"][:, m[1], y0w : y0w + ny, :]
                else:
                    w2sl = s["w2o"][:, m[1], y0w : y0w + ny, :]
                if on_pool[t_i]:
                    P = pperms.tile([128, BLK, O], F16, tag="PP")
                    eng = nc.gpsimd
                else:
                    P = pterms.tile([128, BLK, O], F16, tag="P")
                    eng = nc.vector
                eng.tensor_mul(
                    P[:, y0w : y0w + ny, :].rearrange(
                        "p y (a b) -> p y a b", b=2
                    ),
                    s["ct"][:, i0 + y0w : i0 + y0w + ny, k, :].rearrange(
                        "p y (a b) -> p y a b", b=2
                    ),
                    w2sl.unsqueeze(2).broadcast_to([128, ny, O // 2, 2]),
                )
                ptile[t_i] = P

            def emit_pacc(t_i):
                k, r, s_, c0, c1 = tl[t_i]
                dx = (k % 3 - 1) + s_
                P_f = ptile.pop(t_i)[:, :, :].rearrange("p y o -> p (y o)")
                for cc in range(c0, c1):
                    csl = slice(cc * 512, (cc + 1) * 512)
                    nc.tensor.matmul(
                        pacc_f[:, csl],
                        identh[:, HALO + dx, :],
                        P_f[:, csl],
                        start=(t_i == 0),
                        stop=(t_i == last_touch[cc]),
                    )

            delayed = []
            for p, t_i in enumerate(order):
                emit_pmul(t_i)
                if on_pool[t_i]:
                    delayed.append((p, t_i))
                else:
                    emit_pacc(t_i)
                while delayed and (
                    len(delayed) >= LOOKAHEAD
                    or p - delayed[0][0] >= 2 * LOOKAHEAD
                ):
                    emit_pacc(delayed.pop(0)[1])
                u = next(feeder, None)
                if u is not None:
                    u()
            for _, t_i in delayed:
                emit_pacc(t_i)
            return pacc

        def emit_out(blk, pacc):
            # [x, y, o] -> [(y%2, o), y//2, x] via xbar DMA transpose (fp16),
            # then cast fp32 and store with a matching strided HBM pattern
            by0 = blk * BLK
            S = spool.tile([128, BLK, O], F16, tag="S")
            nc.scalar.copy(out=S, in_=pacc)
            T = outp.tile([128, BLK // 2, W], F16, tag="T")
            nc.sync.dma_start_transpose(
                out=T[:, :, :], in_=S[:, :, :].rearrange("p y o -> p (y o)")
            )
            for h in range(2):
                obuf = outp.tile([128, BLK // 4, W], F32, tag="obuf")
                nc.scalar.copy(
                    out=obuf, in_=T[:, h * (BLK // 4) : (h + 1) * (BLK // 4), :]
                )
                r0 = by0 + h * (BLK // 2)
                for yl in range(2):
                    nc.sync.dma_start(
                        out=out_d[:, r0 + yl : r0 + BLK // 2 : 2, :],
                        in_=obuf[64 * yl : 64 * (yl + 1), :, :],
                    )

        # ---------- pipeline ----------
        for u in setup_units(0):
            u()
        for blk in range(NBLK):
            feeder = iter(setup_units(blk + 1)) if blk + 1 < NBLK else iter(())
            pacc = emit_terms(blk, feeder)
            for u in feeder:
                u()
            emit_out(blk, pacc)
            st.pop(blk, None)


def build_program(b_off, terms):
    slots, biases, term_meta = _field_plan(terms, np.asarray(b_off, np.float32))
    nslot = len(slots)
    nc = bacc.Bacc("TRN2", target_bir_lowering=False, debug=False, num_devices=8)
    x_d = nc.dram_tensor("x", [C, H, W], F16, kind="ExternalInput").ap()
    wk_d = nc.dram_tensor("wk", [C, 9, 18], F16, kind="ExternalInput").ap()
    wf_d = nc.dram_tensor("wf", [C, NTAP * O], F16, kind="ExternalInput").ap()
    identh_d = nc.dram_tensor(
        "identh", [128, 2 * HALO + 1, 128], F16, kind="ExternalInput"
    ).ap()
    ident32_d = nc.dram_tensor(
        "ident32", [128, 128], F32, kind="ExternalInput"
    ).ap()
    btab_d = nc.dram_tensor("btab", [128, nslot], F32, kind="ExternalInput").ap()
    out_d = nc.dram_tensor("out", [O, H, W], F32, kind="ExternalOutput").ap()
    with tile.TileContext(nc) as tc:
        _body(
            tc, nc,
            (x_d, wk_d, wf_d, identh_d, ident32_d, btab_d, out_d),
            terms, term_meta, slots,
        )
    nc.compile()
    return nc


def _host_pack(x_b, w_off, w_dcn, b_off, terms):
    slots, biases, _ = _field_plan(terms, b_off)
    # wk[c, 3*ky+kx, o] = w_off[o, c, ky, kx]
    wk = np.transpose(
        w_off.reshape(18, C, 9), (1, 2, 0)
    ).astype(np.float16)
    # wf[c, 64*(3*ky+kx) + o] = w_dcn[o, c, ky, kx]
    wf = np.transpose(
        w_dcn.reshape(O, C, NTAP), (1, 2, 0)
    ).reshape(C, NTAP * O).astype(np.float16)
    identh = np.zeros((128, 2 * HALO + 1, 128), np.float16)
    for j in range(2 * HALO + 1):
        d = j - HALO
        for mm in range(128):
            if 0 <= mm + d < 128:
                identh[mm + d, j, mm] = 1.0
    ident32 = np.eye(128, dtype=np.float32)
    btab = np.broadcast_to(biases, (128, len(slots))).copy()
    return {
        "wk": wk, "wf": wf,
        "identh": identh, "ident32": ident32, "btab": btab,
    }


def kernel(x, w_off, b_off, w_dcn):
    x = np.ascontiguousarray(x, np.float32)
    w_off = np.ascontiguousarray(w_off, np.float32)
    b_off = np.ascontiguousarray(b_off, np.float32)
    w_dcn = np.ascontiguousarray(w_dcn, np.float32)
    off = _host_offsets(x, w_off, b_off)
    terms = _active_terms(off)
    nc = build_program(b_off, terms)
    const = _host_pack(x, w_off, w_dcn, b_off, terms)
    in_maps = [
        {"x": x[b].astype(np.float16), **const} for b in range(x.shape[0])
    ]
    res = run_bass_kernel_spmd(nc, in_maps, core_ids=list(range(8)))
    global LAST_RESULTS
    LAST_RESULTS = res
    return np.stack([res.results[b]["out"] for b in range(x.shape[0])])


if __name__ == "__main__":
    inp = dict(np.load("/root/problem/inputs.npz"))
    out = kernel(**inp)
    ref = np.load("/root/problem/ref_out.npy")
    err = np.abs(out - ref).max()
    print("absmax err:", err, "rel:", err / np.abs(ref).max())
